# revision 6
# baseline (speedup 1.0000x reference)
"""Single-head causal attention (B=16, S=2048, D=1024, Dh=128) on 8 TRN2 cores.

Sharding: data-parallel over batch — each core computes 2 full batches.
Per-core pipeline (all shapes per batch):
  A: load x [S,D] naturally, PE-transpose 128x128 tiles -> xT [D-part, S]
  B: QT/KT = W.T @ xT (fp32, W-chunk stationary); VT likewise, evicted to fp16;
     bias and the *sqrt(D) score scale folded into the ACT eviction.
  C: per q-tile: scores = QT_i.T @ KT (fp32, PSUM), causal mask add (DVE),
     row max (DVE, negated), exp with -max bias + fused row-sum (ACT -> fp16),
     fp16 xbar DMA transpose of attn tiles, attnT @ V accumulated into the
     same PSUM tile, 1/rowsum fold in ACT eviction, DMA out.
"""

import numpy as np

import concourse.bass as bass
import concourse.mybir as mybir
import concourse.tile as tile
from concourse import bacc

F32 = mybir.dt.float32
F16 = mybir.dt.float16
NEG_BIG = -1e30

B_FULL = 16
S_FULL = 2048
D_FULL = 1024
DH = 128
N_CORES = 8


def attention_body(tc, x, wq, wk, wv, bq, bk, bv, out, *, S, D, scale):
    nc = tc.nc
    NT = S // 128   # number of 128-row seq tiles
    KC = D // 128   # number of 128-row contraction chunks
    NB = x.shape[0]  # batches per core

    with tc.tile_pool(name="const", bufs=1) as const, \
         tc.tile_pool(name="xa", bufs=4) as xa, \
         tc.tile_pool(name="xt", bufs=1) as xtp, \
         tc.tile_pool(name="qk", bufs=1) as qkp, \
         tc.tile_pool(name="vv", bufs=1) as vvp, \
         tc.tile_pool(name="pp", bufs=2) as ppp, \
         tc.tile_pool(name="pt", bufs=2) as ptp, \
         tc.tile_pool(name="oo", bufs=3) as oop, \
         tc.tile_pool(name="stats", bufs=12) as stp:

        # --- constants ---
        ident = const.tile([128, 128], F32)
        nc.gpsimd.memset(ident, 0.0)
        nc.gpsimd.affine_select(
            out=ident, in_=ident, compare_op=mybir.AluOpType.not_equal,
            fill=1.0, base=0, pattern=[[-1, 128]], channel_multiplier=1,
        )
        cmask = const.tile([128, 128], F32)
        nc.gpsimd.memset(cmask, 0.0)
        # keep 0 where q >= k (partition - free >= 0), else NEG_BIG
        nc.gpsimd.affine_select(
            out=cmask, in_=cmask, compare_op=mybir.AluOpType.is_ge,
            fill=NEG_BIG, base=0, pattern=[[-1, 128]], channel_multiplier=1,
        )

        w_sb = []
        for wi, w in enumerate((wq, wk, wv)):
            t = const.tile([128, KC, DH], F32, tag=f"w_sb{wi}")
            nc.sync.dma_start(out=t, in_=w.rearrange("(c p) h -> p c h", p=128))
            w_sb.append(t)
        b_sb = []
        for bi, bv_ap in enumerate((bq, bk, bv)):
            t = const.tile([128, 1], F32, tag=f"b_sb{bi}")
            nc.sync.dma_start(out=t, in_=bv_ap)
            b_sb.append(t)
        bq_scaled = const.tile([128, 1], F32)
        nc.vector.tensor_scalar_mul(bq_scaled, b_sb[0], float(scale))

        for b in range(NB):
            # ---------------- phase A+B: xT and projections ----------------
            xt = xtp.tile([128, KC, S], F32, tag="xt")
            qt = qkp.tile([128, S], F32, tag="qt")
            kt = qkp.tile([128, S], F32, tag="kt")
            vt16 = vvp.tile([128, S], F16, tag="vt16")
            vnat = vvp.tile([128, NT, DH], F16, tag="vnat")

            with tc.tile_pool(name="trps", bufs=4, space="PSUM") as trps, \
                 tc.tile_pool(name="mmps", bufs=4, space="PSUM") as mmps:
                for i in range(NT):
                    xn = xa.tile([128, D], F32, tag="xn")
                    nc.sync.dma_start(out=xn, in_=x[b, i * 128:(i + 1) * 128, :])
                    for cg in range(KC // 4):
                        trt = trps.tile([128, 512], F32, tag="trt")
                        for t in range(4):
                            c = cg * 4 + t
                            nc.tensor.transpose(
                                trt[:, t * 128:(t + 1) * 128],
                                xn[:, c * 128:(c + 1) * 128],
                                ident,
                            )
                        src = trt.rearrange("p (c s) -> p c s", c=4)
                        dst = xt[:, cg * 4:(cg + 1) * 4, i * 128:(i + 1) * 128]
                        # one writer engine per c-chunk so each proj matmul's
                        # rhs waits on a single semaphore
                        if cg % 2 == 0:
                            nc.scalar.copy(dst, src)
                        else:
                            nc.vector.tensor_copy(dst, src)

                # projections: out_chunks of 512 free each get their own psum tile
                proj = (
                    (w_sb[0], bq_scaled, float(scale), qt),
                    (w_sb[1], b_sb[1], 1.0, kt),
                    (w_sb[2], b_sb[2], 1.0, vt16),
                )
                for (wt, bias_ap, sc, dst) in proj:
                    for n in range(S // 512):
                        ps = mmps.tile([128, 512], F32, tag="ps")
                        for c in range(KC):
                            nc.tensor.matmul(
                                ps,
                                lhsT=wt[:, c, :],
                                rhs=xt[:, c, n * 512:(n + 1) * 512],
                                start=(c == 0), stop=(c == KC - 1),
                            )
                        nc.scalar.activation(
                            dst[:, n * 512:(n + 1) * 512], ps,
                            mybir.ActivationFunctionType.Identity,
                            bias=bias_ap, scale=sc,
                        )
                # V to natural [s, dh] layout via fp16 xbar transpose
                for i in range(NT):
                    nc.sync.dma_start(
                        out=vnat[:, i, :], in_=vt16[:, i * 128:(i + 1) * 128],
                        transpose=True,
                    )

            # ---------------- phase C: attention ----------------
            with tc.tile_pool(name="scps", bufs=2, space="PSUM") as scps:
                for i in range(NT):
                    W = (i + 1) * 128
                    sc = scps.tile([128, S], F32, tag="sc")
                    for n in range((W + 511) // 512):
                        wn = min(512, W - n * 512)
                        nc.tensor.matmul(
                            sc[:, n * 512:n * 512 + wn],
                            lhsT=qt[:, i * 128:(i + 1) * 128],
                            rhs=kt[:, n * 512:n * 512 + wn],
                            start=True, stop=True,
                        )
                    # causal mask on the diagonal tile
                    nc.vector.tensor_add(
                        sc[:, i * 128:W], sc[:, i * 128:W], cmask
                    )
                    negm = stp.tile([128, 1], F32, tag="negm")
                    nc.vector.tensor_reduce(
                        negm, sc[:, :W], axis=mybir.AxisListType.X,
                        op=mybir.AluOpType.max, negate=True,
                    )
                    p = ppp.tile([128, S], F16, tag="p")
                    l = stp.tile([128, 1], F32, tag="l")
                    nc.scalar.activation(
                        p[:, :W], sc[:, :W],
                        mybir.ActivationFunctionType.Exp,
                        bias=negm, scale=1.0, accum_out=l,
                    )
                    r = stp.tile([128, 1], F32, tag="r")
                    nc.vector.reciprocal(r, l)
                    ptile = ptp.tile([128, S], F16, tag="ptile")
                    for j in range(i + 1):
                        nc.sync.dma_start(
                            out=ptile[:, j * 128:(j + 1) * 128],
                            in_=p[:, j * 128:(j + 1) * 128],
                            transpose=True,
                        )
                    # attnT @ V accumulated into the scores psum region
                    for j in range(i + 1):
                        nc.tensor.matmul(
                            sc[:, 0:DH],
                            lhsT=ptile[:, j * 128:(j + 1) * 128],
                            rhs=vnat[:, j, :],
                            start=(j == 0), stop=(j == i),
                        )
                    o = oop.tile([128, DH], F32, tag="o")
                    nc.scalar.mul(o, sc[:, 0:DH], r)
                    nc.sync.dma_start(
                        out=out[b, i * 128:(i + 1) * 128, :], in_=o
                    )


def build_attention_nc(nb=2, S=S_FULL, D=D_FULL):
    # Bacc (not raw Bass): its compile() pass legalizes sync for this
    # toolchain (≤1 wait per instruction, waits moved to ldweights/events).
    nc = bacc.Bacc(trn_type="TRN2")
    x_h = nc.dram_tensor("x", [nb, S, D], F32, kind="ExternalInput")
    wq_h = nc.dram_tensor("Wq", [D, DH], F32, kind="ExternalInput")
    wk_h = nc.dram_tensor("Wk", [D, DH], F32, kind="ExternalInput")
    wv_h = nc.dram_tensor("Wv", [D, DH], F32, kind="ExternalInput")
    bq_h = nc.dram_tensor("bq", [DH, 1], F32, kind="ExternalInput")
    bk_h = nc.dram_tensor("bk", [DH, 1], F32, kind="ExternalInput")
    bv_h = nc.dram_tensor("bv", [DH, 1], F32, kind="ExternalInput")
    out_h = nc.dram_tensor("out", [nb, S, DH], F32, kind="ExternalOutput")
    with tile.TileContext(nc) as tc:
        attention_body(
            tc, x_h.ap(), wq_h.ap(), wk_h.ap(), wv_h.ap(),
            bq_h.ap(), bk_h.ap(), bv_h.ap(), out_h.ap(),
            S=S, D=D, scale=float(D) ** 0.5,
        )
    nc.compile()
    return nc


_NC_CACHE = {}


def _get_nc():
    if "nc" not in _NC_CACHE:
        _NC_CACHE["nc"] = build_attention_nc()
    return _NC_CACHE["nc"]


def make_in_maps(x, Wq, bq, Wk, bk, Wv, bv):
    x = np.ascontiguousarray(np.asarray(x, dtype=np.float32))
    args = {
        "Wq": np.ascontiguousarray(np.asarray(Wq, np.float32)),
        "Wk": np.ascontiguousarray(np.asarray(Wk, np.float32)),
        "Wv": np.ascontiguousarray(np.asarray(Wv, np.float32)),
        "bq": np.ascontiguousarray(np.asarray(bq, np.float32).reshape(DH, 1)),
        "bk": np.ascontiguousarray(np.asarray(bk, np.float32).reshape(DH, 1)),
        "bv": np.ascontiguousarray(np.asarray(bv, np.float32).reshape(DH, 1)),
    }
    nb = x.shape[0] // N_CORES
    return [
        {"x": x[c * nb:(c + 1) * nb], **args} for c in range(N_CORES)
    ]


def kernel(x, Wq, bq, Wk, bk, Wv, bv):
    from concourse.bass_utils import run_bass_kernel_spmd

    nc = _get_nc()
    in_maps = make_in_maps(x, Wq, bq, Wk, bk, Wv, bv)
    res = run_bass_kernel_spmd(nc, in_maps, core_ids=list(range(N_CORES)))
    return np.concatenate([r["out"] for r in res.results], axis=0)
